# revision 11
# baseline (speedup 1.0000x reference)
"""Trainium2 Bass kernel for fused MoE expert-reduction + residual add + RMSNorm.

Computes (matching the jax reference):
    expert_reduction[t,h] = sum_e scale[e,t] * active[e,t,h]
    output_residual       = expert_reduction + token_input + residual
    hidden_states         = output_residual * rsqrt(mean(output_residual^2, -1) + eps) * norm_weight
Returns (hidden_states, output_residual), both [T, H] float32.

Strategy: shard the token dim T=4096 across 8 NeuronCores (512 tokens each).
Each core runs an identical fully-unrolled Tile program over 4 tiles of 128
tokens x 4096 hidden:
  - expert reduction as a chain of 8 fused scalar_tensor_tensor multiply-adds
    on the Vector engine (per-partition scalar = per-token expert scale),
    with the (token_input + residual) sum folded in as the chain seed,
  - RMS statistic via one fused Square+row-accumulate on the Scalar engine,
  - final scaling as one fused (x * rsqrt) * weight scalar_tensor_tensor.
All DMAs are 2 MiB contiguous 128-partition transfers (peak HBM bandwidth).
"""

import numpy as np

import concourse.bass as bass
import concourse.mybir as mybir
import concourse.tile as tile
from concourse.bass_utils import run_bass_kernel_spmd

# ---------------------------------------------------------------------------
# Workaround for walrus "Too many sync wait commands": this neuronxcc build
# accepts only ONE embedded sync wait per instruction (any encoding), but
# Tile's wait-assignment pass attaches one wait per outstanding producer
# semaphore. Post-pass over the scheduled module: for every instruction
# carrying >1 waits, keep the first and move each extra wait onto its own
# same-engine NOP inserted immediately before the instruction.
_wsplit_counter = [0]


def _split_embedded_waits(nc, max_waits: int = 1) -> int:
    n_split = 0
    for f in nc.m.functions:
        for b in f.blocks:
            new_insts = []
            changed = False
            for inst in b.instructions:
                si = inst.sync_info
                if si is not None and len(si.on_wait) > max_waits:
                    waits = list(si.on_wait)
                    keep, extra = waits[:max_waits], waits[max_waits:]
                    for w in extra:
                        _wsplit_counter[0] += 1
                        nop = mybir.InstNoOp(
                            name=f"WSPLIT-{_wsplit_counter[0]}", ins=[], outs=[]
                        )
                        nop.engine = inst.engine
                        nop.sync_info = mybir.SyncInfo(on_wait=[w], on_update=[])
                        new_insts.append(nop)
                        n_split += 1
                    inst.sync_info = mybir.SyncInfo(
                        on_wait=keep, on_update=list(si.on_update)
                    )
                    changed = True
                new_insts.append(inst)
            if changed:
                b.instructions = new_insts
    return n_split
# ---------------------------------------------------------------------------

E = 8          # experts
T = 4096       # tokens (full)
H = 4096       # hidden
N_CORES = 8
TL = T // N_CORES   # tokens per core = 512
P = 128             # SBUF partitions
NT = TL // P        # token tiles per core = 4

F32 = mybir.dt.float32


def _build_nc(reps: int = 1) -> bass.Bass:
    """reps>1 repeats the whole per-core program inside one NEFF (same I/O,
    identical results) — used only for timing, to amortize dispatch overhead."""
    nc = bass.Bass()

    active = nc.declare_dram_parameter("active", [E, TL, H], F32, isOutput=False)
    token = nc.declare_dram_parameter("token", [TL, H], F32, isOutput=False)
    residual = nc.declare_dram_parameter("residual", [TL, H], F32, isOutput=False)
    scale_t = nc.declare_dram_parameter("scale_t", [TL, E], F32, isOutput=False)
    w_bcast = nc.declare_dram_parameter("w_bcast", [P, H], F32, isOutput=False)
    eps_col = nc.declare_dram_parameter("eps_col", [P, 1], F32, isOutput=False)
    out_res = nc.declare_dram_parameter("out_res", [TL, H], F32, isOutput=True)
    hidden = nc.declare_dram_parameter("hidden", [TL, H], F32, isOutput=True)

    mult = mybir.AluOpType.mult
    add = mybir.AluOpType.add

    with tile.TileContext(nc) as tc:
        with (
            tc.tile_pool(name="consts", bufs=1) as consts,
            tc.tile_pool(name="act", bufs=3) as act_pool,
            tc.tile_pool(name="tok", bufs=2) as tok_pool,
            tc.tile_pool(name="res", bufs=2) as res_pool,
            tc.tile_pool(name="acc", bufs=2) as acc_pool,
            tc.tile_pool(name="hid", bufs=1) as hid_pool,
            tc.tile_pool(name="stats", bufs=2) as stats_pool,
        ):
            w_tile = consts.tile([P, H], F32, tag="w")
            nc.sync.dma_start(w_tile[:], w_bcast[:])
            eps_tile = consts.tile([P, 1], F32, tag="eps")
            nc.sync.dma_start(eps_tile[:], eps_col[:])

            for rep_it in range(reps * NT):
                it = rep_it % NT
                trow = bass.ts(it, P)

                s_tile = stats_pool.tile([P, E], F32, tag="scale")
                nc.sync.dma_start(s_tile[:], scale_t[trow, :])
                tok = tok_pool.tile([P, H], F32, tag="tok")
                nc.sync.dma_start(tok[:], token[trow, :])
                res = res_pool.tile([P, H], F32, tag="res")
                nc.sync.dma_start(res[:], residual[trow, :])

                # tr = token + residual (in place into tok)
                nc.vector.tensor_add(tok[:], tok[:], res[:])

                # acc = sum_e scale[e] * active[e]  + tr, as a chain of fused
                # multiply-adds: acc = active_e * s_e + prev
                acc = acc_pool.tile([P, H], F32, tag="acc")
                for e in range(E):
                    a_t = act_pool.tile([P, H], F32, tag="a")
                    nc.sync.dma_start(a_t[:], active[e, trow, :])
                    prev = tok if e == 0 else acc
                    nc.vector.scalar_tensor_tensor(
                        out=acc[:],
                        in0=a_t[:],
                        scalar=s_tile[:, e : e + 1],
                        in1=prev[:],
                        op0=mult,
                        op1=add,
                    )

                nc.sync.dma_start(out_res[trow, :], acc[:])

                # sumsq[p] = sum_h acc[p,h]^2 (Scalar engine, fused accumulate).
                # hid is dead until the final scaling op, so use it as the
                # Square scratch output.
                hid = hid_pool.tile([P, H], F32, tag="hid")
                sumsq = stats_pool.tile([P, 1], F32, tag="sumsq")
                nc.scalar.activation(
                    hid[:],
                    acc[:],
                    mybir.ActivationFunctionType.Square,
                    accum_out=sumsq[:],
                )
                # varps = sumsq/H + eps
                varps = stats_pool.tile([P, 1], F32, tag="varps")
                nc.vector.scalar_tensor_tensor(
                    out=varps[:],
                    in0=sumsq[:],
                    scalar=float(1.0 / H),
                    in1=eps_tile[:],
                    op0=mult,
                    op1=add,
                )
                # rmsinv = sqrt(1/varps)
                recip = stats_pool.tile([P, 1], F32, tag="recip")
                nc.vector.reciprocal(recip[:], varps[:])
                rmsinv = stats_pool.tile([P, 1], F32, tag="rmsinv")
                nc.scalar.activation(
                    rmsinv[:], recip[:], mybir.ActivationFunctionType.Sqrt
                )

                # hidden = (acc * rmsinv) * w
                nc.vector.scalar_tensor_tensor(
                    out=hid[:],
                    in0=acc[:],
                    scalar=rmsinv[:],
                    in1=w_tile[:],
                    op0=mult,
                    op1=mult,
                )
                nc.sync.dma_start(hidden[trow, :], hid[:])

    return nc


_NC_CACHE: list = []


def _get_nc() -> bass.Bass:
    if not _NC_CACHE:
        nc = _build_nc()
        _split_embedded_waits(nc)
        _NC_CACHE.append(nc)
    return _NC_CACHE[0]


def _run(in_maps, trace=False, **kwargs):
    nc = _get_nc()
    return run_bass_kernel_spmd(
        nc, in_maps, core_ids=list(range(N_CORES)), trace=trace, **kwargs
    )


def make_in_maps(
    active, token, residual, scale, w, eps
) -> list[dict[str, np.ndarray]]:
    w_b = np.ascontiguousarray(
        np.broadcast_to(np.asarray(w, np.float32)[None, :], (P, H))
    )
    eps_c = np.full((P, 1), float(eps), np.float32)
    in_maps = []
    for c in range(N_CORES):
        sl = slice(c * TL, (c + 1) * TL)
        in_maps.append(
            {
                "active": np.ascontiguousarray(active[:, sl, :]),
                "token": np.ascontiguousarray(token[sl]),
                "residual": np.ascontiguousarray(residual[sl]),
                "scale_t": np.ascontiguousarray(scale[:, sl].T),
                "w_bcast": w_b,
                "eps_col": eps_c,
            }
        )
    return in_maps


def kernel(
    residual,
    norm_weight,
    device_num_experts,
    scale_input,
    active_experts_token_input,
    token_input,
    eps,
):
    active = np.asarray(active_experts_token_input, np.float32)
    token = np.asarray(token_input, np.float32)
    res = np.asarray(residual, np.float32)
    scale = np.asarray(scale_input, np.float32)
    w = np.asarray(norm_weight, np.float32)
    eps_v = float(np.asarray(eps))
    assert active.shape == (E, T, H), active.shape

    in_maps = make_in_maps(active, token, res, scale, w, eps_v)
    results = _run(in_maps).results

    hidden = np.concatenate([r["hidden"] for r in results], axis=0)
    out_res = np.concatenate([r["out_res"] for r in results], axis=0)
    return hidden, out_res


# revision 14
# speedup vs baseline: 1.0312x; 1.0312x over previous
"""Trainium2 Bass kernel for fused MoE expert-reduction + residual add + RMSNorm.

Computes (matching the jax reference):
    expert_reduction[t,h] = sum_e scale[e,t] * active[e,t,h]
    output_residual       = expert_reduction + token_input + residual
    hidden_states         = output_residual * rsqrt(mean(output_residual^2, -1) + eps) * norm_weight
Returns (hidden_states, output_residual), both [T, H] float32.

Strategy: shard the token dim T=4096 across 8 NeuronCores (512 tokens each).
Each core runs an identical fully-unrolled Tile program over 4 tiles of 128
tokens x 4096 hidden:
  - expert reduction as a chain of 8 fused scalar_tensor_tensor multiply-adds
    on the Vector engine (per-partition scalar = per-token expert scale),
    with the (token_input + residual) sum folded in as the chain seed,
  - RMS statistic via one fused Square+row-accumulate on the Scalar engine,
  - final scaling as one fused (x * rsqrt) * weight scalar_tensor_tensor.
All DMAs are 2 MiB contiguous 128-partition transfers (peak HBM bandwidth).
"""

import numpy as np

import concourse.bass as bass
import concourse.mybir as mybir
import concourse.tile as tile
from concourse.bass_utils import run_bass_kernel_spmd

# ---------------------------------------------------------------------------
# Workaround for walrus "Too many sync wait commands": this neuronxcc build
# accepts only ONE embedded sync wait per instruction (any encoding), but
# Tile's wait-assignment pass attaches one wait per outstanding producer
# semaphore. Post-pass over the scheduled module: for every instruction
# carrying >1 waits, keep the first and move each extra wait onto its own
# same-engine NOP inserted immediately before the instruction.
_wsplit_counter = [0]


def _split_embedded_waits(nc, max_waits: int = 1) -> int:
    n_split = 0
    for f in nc.m.functions:
        for b in f.blocks:
            new_insts = []
            changed = False
            for inst in b.instructions:
                si = inst.sync_info
                if si is not None and len(si.on_wait) > max_waits:
                    waits = list(si.on_wait)
                    keep, extra = waits[:max_waits], waits[max_waits:]
                    for w in extra:
                        _wsplit_counter[0] += 1
                        nop = mybir.InstNoOp(
                            name=f"WSPLIT-{_wsplit_counter[0]}", ins=[], outs=[]
                        )
                        nop.engine = inst.engine
                        nop.sync_info = mybir.SyncInfo(on_wait=[w], on_update=[])
                        new_insts.append(nop)
                        n_split += 1
                    inst.sync_info = mybir.SyncInfo(
                        on_wait=keep, on_update=list(si.on_update)
                    )
                    changed = True
                new_insts.append(inst)
            if changed:
                b.instructions = new_insts
    return n_split
# ---------------------------------------------------------------------------

E = 8          # experts
T = 4096       # tokens (full)
H = 4096       # hidden
N_CORES = 8
TL = T // N_CORES   # tokens per core = 512
P = 128             # SBUF partitions
NT = TL // P        # token tiles per core = 4

F32 = mybir.dt.float32


def _build_nc(reps: int = 1) -> bass.Bass:
    """reps>1 repeats the whole per-core program inside one NEFF (same I/O,
    identical results) — used only for timing, to amortize dispatch overhead."""
    nc = bass.Bass()

    active = nc.declare_dram_parameter("active", [E, TL, H], F32, isOutput=False)
    token = nc.declare_dram_parameter("token", [TL, H], F32, isOutput=False)
    residual = nc.declare_dram_parameter("residual", [TL, H], F32, isOutput=False)
    scale_all = nc.declare_dram_parameter("scale_all", [P, NT * E], F32, isOutput=False)
    w_row = nc.declare_dram_parameter("w_row", [1, H], F32, isOutput=False)
    eps_col = nc.declare_dram_parameter("eps_col", [P, 1], F32, isOutput=False)
    out_res = nc.declare_dram_parameter("out_res", [TL, H], F32, isOutput=True)
    hidden = nc.declare_dram_parameter("hidden", [TL, H], F32, isOutput=True)

    mult = mybir.AluOpType.mult
    add = mybir.AluOpType.add
    CH = 512  # PSUM bank width (fp32)

    with tile.TileContext(nc) as tc:
        with (
            tc.tile_pool(name="consts", bufs=1) as consts,
            tc.tile_pool(name="psum", bufs=2, space="PSUM") as psum_pool,
            tc.tile_pool(name="act", bufs=3) as act_pool,
            tc.tile_pool(name="tok", bufs=2) as tok_pool,
            tc.tile_pool(name="res", bufs=2) as res_pool,
            tc.tile_pool(name="acc", bufs=2) as acc_pool,
            tc.tile_pool(name="hid", bufs=1) as hid_pool,
            tc.tile_pool(name="stats", bufs=2) as stats_pool,
        ):
            # Broadcast norm_weight [1,H] -> [P,H] on-chip: K=1 matmul with a
            # ones column (out[i,j] = 1 * w[0,j]), one PSUM bank per H-chunk,
            # copied to SBUF on the Scalar engine. Reads 16 KB from HBM
            # instead of a 2 MiB pre-broadcast tensor.
            wr_tile = consts.tile([1, H], F32, tag="wr")
            nc.sync.dma_start(wr_tile[:], w_row[:])
            ones_tile = consts.tile([1, P], F32, tag="ones")
            nc.vector.memset(ones_tile[:], 1.0)
            w_tile = consts.tile([P, H], F32, tag="w")
            for c in range(H // CH):
                pt = psum_pool.tile([P, CH], F32, tag="wb")
                nc.tensor.matmul(
                    pt[:],
                    ones_tile[:],
                    wr_tile[:, bass.ts(c, CH)],
                    start=True,
                    stop=True,
                )
                nc.scalar.copy(w_tile[:, bass.ts(c, CH)], pt[:])

            eps_tile = consts.tile([P, 1], F32, tag="eps")
            nc.sync.dma_start(eps_tile[:], eps_col[:])

            # All NT scale tiles in one 16 KB DMA: [P, NT*E], col = it*E + e
            # (host pre-transposes: scale_all[p, it*E+e] = scale[e, it*P+p]).
            s_all = consts.tile([P, NT * E], F32, tag="s_all")
            nc.sync.dma_start(s_all[:], scale_all[:])

            for rep_it in range(reps * NT):
                it = rep_it % NT
                trow = bass.ts(it, P)

                s_tile = s_all[:, it * E : (it + 1) * E]
                tok = tok_pool.tile([P, H], F32, tag="tok")
                nc.sync.dma_start(tok[:], token[trow, :])
                res = res_pool.tile([P, H], F32, tag="res")
                nc.sync.dma_start(res[:], residual[trow, :])

                # tr = token + residual (in place into tok)
                nc.vector.tensor_add(tok[:], tok[:], res[:])

                # acc = sum_e scale[e] * active[e]  + tr, as a chain of fused
                # multiply-adds: acc = active_e * s_e + prev
                acc = acc_pool.tile([P, H], F32, tag="acc")
                for e in range(E):
                    a_t = act_pool.tile([P, H], F32, tag="a")
                    nc.sync.dma_start(a_t[:], active[e, trow, :])
                    prev = tok if e == 0 else acc
                    nc.vector.scalar_tensor_tensor(
                        out=acc[:],
                        in0=a_t[:],
                        scalar=s_tile[:, e : e + 1],
                        in1=prev[:],
                        op0=mult,
                        op1=add,
                    )

                nc.sync.dma_start(out_res[trow, :], acc[:])

                # sumsq[p] = sum_h acc[p,h]^2 (Scalar engine, fused accumulate).
                # hid is dead until the final scaling op, so use it as the
                # Square scratch output.
                hid = hid_pool.tile([P, H], F32, tag="hid")
                sumsq = stats_pool.tile([P, 1], F32, tag="sumsq")
                nc.scalar.activation(
                    hid[:],
                    acc[:],
                    mybir.ActivationFunctionType.Square,
                    accum_out=sumsq[:],
                )
                # varps = sumsq/H + eps
                varps = stats_pool.tile([P, 1], F32, tag="varps")
                nc.vector.scalar_tensor_tensor(
                    out=varps[:],
                    in0=sumsq[:],
                    scalar=float(1.0 / H),
                    in1=eps_tile[:],
                    op0=mult,
                    op1=add,
                )
                # rmsinv = sqrt(1/varps)
                recip = stats_pool.tile([P, 1], F32, tag="recip")
                nc.vector.reciprocal(recip[:], varps[:])
                rmsinv = stats_pool.tile([P, 1], F32, tag="rmsinv")
                nc.scalar.activation(
                    rmsinv[:], recip[:], mybir.ActivationFunctionType.Sqrt
                )

                # hidden = (acc * rmsinv) * w
                nc.vector.scalar_tensor_tensor(
                    out=hid[:],
                    in0=acc[:],
                    scalar=rmsinv[:],
                    in1=w_tile[:],
                    op0=mult,
                    op1=mult,
                )
                nc.sync.dma_start(hidden[trow, :], hid[:])

    return nc


_NC_CACHE: list = []


def _get_nc() -> bass.Bass:
    if not _NC_CACHE:
        nc = _build_nc()
        _split_embedded_waits(nc)
        _NC_CACHE.append(nc)
    return _NC_CACHE[0]


def _run(in_maps, trace=False, **kwargs):
    nc = _get_nc()
    return run_bass_kernel_spmd(
        nc, in_maps, core_ids=list(range(N_CORES)), trace=trace, **kwargs
    )


def make_in_maps(
    active, token, residual, scale, w, eps
) -> list[dict[str, np.ndarray]]:
    w_r = np.ascontiguousarray(np.asarray(w, np.float32)[None, :])
    eps_c = np.full((P, 1), float(eps), np.float32)
    in_maps = []
    for c in range(N_CORES):
        sl = slice(c * TL, (c + 1) * TL)
        in_maps.append(
            {
                "active": np.ascontiguousarray(active[:, sl, :]),
                "token": np.ascontiguousarray(token[sl]),
                "residual": np.ascontiguousarray(residual[sl]),
                "scale_all": np.ascontiguousarray(
                    scale[:, sl].reshape(E, NT, P).transpose(2, 1, 0).reshape(P, NT * E)
                ),
                "w_row": w_r,
                "eps_col": eps_c,
            }
        )
    return in_maps


def kernel(
    residual,
    norm_weight,
    device_num_experts,
    scale_input,
    active_experts_token_input,
    token_input,
    eps,
):
    active = np.asarray(active_experts_token_input, np.float32)
    token = np.asarray(token_input, np.float32)
    res = np.asarray(residual, np.float32)
    scale = np.asarray(scale_input, np.float32)
    w = np.asarray(norm_weight, np.float32)
    eps_v = float(np.asarray(eps))
    assert active.shape == (E, T, H), active.shape

    in_maps = make_in_maps(active, token, res, scale, w, eps_v)
    results = _run(in_maps).results

    hidden = np.concatenate([r["hidden"] for r in results], axis=0)
    out_res = np.concatenate([r["out_res"] for r in results], axis=0)
    return hidden, out_res


# revision 16
# speedup vs baseline: 1.2065x; 1.1700x over previous
"""Trainium2 Bass kernel for fused MoE expert-reduction + residual add + RMSNorm.

Computes (matching the jax reference):
    expert_reduction[t,h] = sum_e scale[e,t] * active[e,t,h]
    output_residual       = expert_reduction + token_input + residual
    hidden_states         = output_residual * rsqrt(mean(output_residual^2, -1) + eps) * norm_weight
Returns (hidden_states, output_residual), both [T, H] float32.

Strategy: shard the token dim T=4096 across 8 NeuronCores (512 tokens each).
Each core runs an identical fully-unrolled Tile program over 4 tiles of 128
tokens x 4096 hidden:
  - expert reduction as a chain of 8 fused scalar_tensor_tensor multiply-adds
    on the Vector engine (per-partition scalar = per-token expert scale),
    with the (token_input + residual) sum folded in as the chain seed,
  - RMS statistic via one fused Square+row-accumulate on the Scalar engine,
  - final scaling as one fused (x * rsqrt) * weight scalar_tensor_tensor,
  - norm_weight broadcast [1,H]->[128,H] built on-chip with a K=1 TensorE
    matmul against a ones column (reads 16 KB instead of 2 MiB),
  - all per-token scales in one 16 KB DMA, host pre-transposed.
All large DMAs are 2 MiB contiguous 128-partition transfers (peak HBM
bandwidth). The kernel is HBM-bound: ~96 MiB of traffic per core ≈ 281 us
at the ~358 GB/s per-core HBM limit; measured ~265-300 us steady-state.
"""

import numpy as np

import concourse.bass as bass
import concourse.mybir as mybir
import concourse.tile as tile
from concourse.bass_utils import run_bass_kernel_spmd

# ---------------------------------------------------------------------------
# Workaround for walrus "Too many sync wait commands": this neuronxcc build
# accepts only ONE embedded sync wait per instruction (any encoding), but
# Tile's wait-assignment pass attaches one wait per outstanding producer
# semaphore. Post-pass over the scheduled module: for every instruction
# carrying >1 waits, keep the first and move each extra wait onto its own
# same-engine NOP inserted immediately before the instruction.
_wsplit_counter = [0]


def _split_embedded_waits(nc, max_waits: int = 1) -> int:
    n_split = 0
    for f in nc.m.functions:
        for b in f.blocks:
            new_insts = []
            changed = False
            for inst in b.instructions:
                si = inst.sync_info
                if si is not None and len(si.on_wait) > max_waits:
                    waits = list(si.on_wait)
                    keep, extra = waits[:max_waits], waits[max_waits:]
                    for w in extra:
                        _wsplit_counter[0] += 1
                        nop = mybir.InstNoOp(
                            name=f"WSPLIT-{_wsplit_counter[0]}", ins=[], outs=[]
                        )
                        nop.engine = inst.engine
                        nop.sync_info = mybir.SyncInfo(on_wait=[w], on_update=[])
                        new_insts.append(nop)
                        n_split += 1
                    inst.sync_info = mybir.SyncInfo(
                        on_wait=keep, on_update=list(si.on_update)
                    )
                    changed = True
                new_insts.append(inst)
            if changed:
                b.instructions = new_insts
    return n_split
# ---------------------------------------------------------------------------

E = 8          # experts
T = 4096       # tokens (full)
H = 4096       # hidden
N_CORES = 8
TL = T // N_CORES   # tokens per core = 512
P = 128             # SBUF partitions
NT = TL // P        # token tiles per core = 4

F32 = mybir.dt.float32


def _build_nc(reps: int = 1) -> bass.Bass:
    """reps>1 repeats the whole per-core program inside one NEFF (same I/O,
    identical results) — used only for timing, to amortize dispatch overhead."""
    nc = bass.Bass()

    active = nc.declare_dram_parameter("active", [E, TL, H], F32, isOutput=False)
    token = nc.declare_dram_parameter("token", [TL, H], F32, isOutput=False)
    residual = nc.declare_dram_parameter("residual", [TL, H], F32, isOutput=False)
    scale_all = nc.declare_dram_parameter("scale_all", [P, NT * E], F32, isOutput=False)
    w_row = nc.declare_dram_parameter("w_row", [1, H], F32, isOutput=False)
    eps_col = nc.declare_dram_parameter("eps_col", [P, 1], F32, isOutput=False)
    out_res = nc.declare_dram_parameter("out_res", [TL, H], F32, isOutput=True)
    hidden = nc.declare_dram_parameter("hidden", [TL, H], F32, isOutput=True)

    mult = mybir.AluOpType.mult
    add = mybir.AluOpType.add
    CH = 512  # PSUM bank width (fp32)

    with tile.TileContext(nc) as tc:
        with (
            tc.tile_pool(name="consts", bufs=1) as consts,
            tc.tile_pool(name="psum", bufs=2, space="PSUM") as psum_pool,
            tc.tile_pool(name="act", bufs=3) as act_pool,
            tc.tile_pool(name="tok", bufs=2) as tok_pool,
            tc.tile_pool(name="res", bufs=2) as res_pool,
            tc.tile_pool(name="acc", bufs=2) as acc_pool,
            tc.tile_pool(name="hid", bufs=1) as hid_pool,
            tc.tile_pool(name="stats", bufs=2) as stats_pool,
        ):
            # Broadcast norm_weight [1,H] -> [P,H] on-chip: K=1 matmul with a
            # ones column (out[i,j] = 1 * w[0,j]), one PSUM bank per H-chunk,
            # copied to SBUF on the Scalar engine. Reads 16 KB from HBM
            # instead of a 2 MiB pre-broadcast tensor.
            wr_tile = consts.tile([1, H], F32, tag="wr")
            nc.sync.dma_start(wr_tile[:], w_row[:])
            ones_tile = consts.tile([1, P], F32, tag="ones")
            nc.vector.memset(ones_tile[:], 1.0)
            w_tile = consts.tile([P, H], F32, tag="w")
            for c in range(H // CH):
                pt = psum_pool.tile([P, CH], F32, tag="wb")
                nc.tensor.matmul(
                    pt[:],
                    ones_tile[:],
                    wr_tile[:, bass.ts(c, CH)],
                    start=True,
                    stop=True,
                )
                nc.scalar.copy(w_tile[:, bass.ts(c, CH)], pt[:])

            eps_tile = consts.tile([P, 1], F32, tag="eps")
            nc.sync.dma_start(eps_tile[:], eps_col[:])

            # All NT scale tiles in one 16 KB DMA: [P, NT*E], col = it*E + e
            # (host pre-transposes: scale_all[p, it*E+e] = scale[e, it*P+p]).
            s_all = consts.tile([P, NT * E], F32, tag="s_all")
            nc.sync.dma_start(s_all[:], scale_all[:])

            for rep_it in range(reps * NT):
                it = rep_it % NT
                trow = bass.ts(it, P)

                s_tile = s_all[:, it * E : (it + 1) * E]
                tok = tok_pool.tile([P, H], F32, tag="tok")
                nc.sync.dma_start(tok[:], token[trow, :])
                res = res_pool.tile([P, H], F32, tag="res")
                nc.sync.dma_start(res[:], residual[trow, :])

                # tr = token + residual (in place into tok)
                nc.vector.tensor_add(tok[:], tok[:], res[:])

                # acc = sum_e scale[e] * active[e]  + tr, as a chain of fused
                # multiply-adds: acc = active_e * s_e + prev
                acc = acc_pool.tile([P, H], F32, tag="acc")
                for e in range(E):
                    a_t = act_pool.tile([P, H], F32, tag="a")
                    nc.sync.dma_start(a_t[:], active[e, trow, :])
                    prev = tok if e == 0 else acc
                    nc.vector.scalar_tensor_tensor(
                        out=acc[:],
                        in0=a_t[:],
                        scalar=s_tile[:, e : e + 1],
                        in1=prev[:],
                        op0=mult,
                        op1=add,
                    )

                nc.sync.dma_start(out_res[trow, :], acc[:])

                # sumsq[p] = sum_h acc[p,h]^2 (Scalar engine, fused accumulate).
                # hid is dead until the final scaling op, so use it as the
                # Square scratch output.
                hid = hid_pool.tile([P, H], F32, tag="hid")
                sumsq = stats_pool.tile([P, 1], F32, tag="sumsq")
                nc.scalar.activation(
                    hid[:],
                    acc[:],
                    mybir.ActivationFunctionType.Square,
                    accum_out=sumsq[:],
                )
                # varps = sumsq/H + eps
                varps = stats_pool.tile([P, 1], F32, tag="varps")
                nc.vector.scalar_tensor_tensor(
                    out=varps[:],
                    in0=sumsq[:],
                    scalar=float(1.0 / H),
                    in1=eps_tile[:],
                    op0=mult,
                    op1=add,
                )
                # rmsinv = sqrt(1/varps)
                recip = stats_pool.tile([P, 1], F32, tag="recip")
                nc.vector.reciprocal(recip[:], varps[:])
                rmsinv = stats_pool.tile([P, 1], F32, tag="rmsinv")
                nc.scalar.activation(
                    rmsinv[:], recip[:], mybir.ActivationFunctionType.Sqrt
                )

                # hidden = (acc * rmsinv) * w
                nc.vector.scalar_tensor_tensor(
                    out=hid[:],
                    in0=acc[:],
                    scalar=rmsinv[:],
                    in1=w_tile[:],
                    op0=mult,
                    op1=mult,
                )
                nc.sync.dma_start(hidden[trow, :], hid[:])

    return nc


_NC_CACHE: list = []


def _get_nc() -> bass.Bass:
    if not _NC_CACHE:
        nc = _build_nc()
        _split_embedded_waits(nc)
        _NC_CACHE.append(nc)
    return _NC_CACHE[0]


def _run(in_maps, trace=False, **kwargs):
    nc = _get_nc()
    return run_bass_kernel_spmd(
        nc, in_maps, core_ids=list(range(N_CORES)), trace=trace, **kwargs
    )


def make_in_maps(
    active, token, residual, scale, w, eps
) -> list[dict[str, np.ndarray]]:
    w_r = np.ascontiguousarray(np.asarray(w, np.float32)[None, :])
    eps_c = np.full((P, 1), float(eps), np.float32)
    in_maps = []
    for c in range(N_CORES):
        sl = slice(c * TL, (c + 1) * TL)
        in_maps.append(
            {
                "active": np.ascontiguousarray(active[:, sl, :]),
                "token": np.ascontiguousarray(token[sl]),
                "residual": np.ascontiguousarray(residual[sl]),
                "scale_all": np.ascontiguousarray(
                    scale[:, sl].reshape(E, NT, P).transpose(2, 1, 0).reshape(P, NT * E)
                ),
                "w_row": w_r,
                "eps_col": eps_c,
            }
        )
    return in_maps


def kernel(
    residual,
    norm_weight,
    device_num_experts,
    scale_input,
    active_experts_token_input,
    token_input,
    eps,
):
    active = np.asarray(active_experts_token_input, np.float32)
    token = np.asarray(token_input, np.float32)
    res = np.asarray(residual, np.float32)
    scale = np.asarray(scale_input, np.float32)
    w = np.asarray(norm_weight, np.float32)
    eps_v = float(np.asarray(eps))
    assert active.shape == (E, T, H), active.shape

    in_maps = make_in_maps(active, token, res, scale, w, eps_v)
    try:
        results = _run(in_maps).results
    except Exception:
        # Rare transient NRT_EXEC_UNIT_UNRECOVERABLE wedge right after heavy
        # dispatch sessions — one retry normally succeeds.
        results = _run(in_maps).results

    hidden = np.concatenate([r["hidden"] for r in results], axis=0)
    out_res = np.concatenate([r["out_res"] for r in results], axis=0)
    return hidden, out_res
